# revision 12
# baseline (speedup 1.0000x reference)
"""TransformerXL relative attention on 8 TRN2 NeuronCores (batch-parallel).

v3: fully fused single-pass pipeline, rel_shift via skewed SBUF->SBUF DMA
(no DRAM round trip), PE kept HAM-warm end to end.

Per-core (one batch element):
  - warmup matmul burst releases the HAM clock gate before real work
  - projections (rT, qcb/qpb, kT, v) as 8-matmul PSUM waves; kT and v are
    interleaved with the attention pair pipeline
  - position logits P per (pair, head, q-tile) -> PSUM -> bf16 pst ring
    (rows padded to 1536 with -30000); rel_shift read back with a skewed
    flat AP (partition stride = row_pitch - 1) SBUF->SBUF
  - scores = content matmul + identity-matmul(P_shifted); Exp on ACT with
    accum_out denominators; batched reciprocals; normalize on DVE;
    attn transposed via DMA-xbar (sync queue) to attnT [r, q]
  - PV matmuls (col-packed head pairs), output projection tail
"""

import sys

if "/opt/trn_rl_repo" not in sys.path:
    sys.path.insert(0, "/opt/trn_rl_repo")

import numpy as np

B, Q, MEM, D, H, S = 8, 512, 512, 1024, 16, 64
R = Q + MEM  # 1024
PIT = 1536  # padded row pitch of a pst slot (1024 data + 512 pad)
PAD_VAL = -30000.0
NKD = D // 128  # 8 contraction tiles
NI = D // 128  # 8 hs-tiles
NQT = Q // 128  # 4 q-tiles
NRT = R // 128  # 8 r-tiles
NP = H // 2  # 8 head pairs
NSLOT = 4  # pst ring slots

_CACHE = {}


def _build_nc():
    import concourse.bass as bass_mod
    import concourse.mybir as mybir
    import concourse.tile as tile
    from concourse import bacc
    from concourse.bass import ds
    from concourse.masks import make_identity

    f32 = mybir.dt.float32
    bf16 = mybir.dt.bfloat16
    AF = mybir.ActivationFunctionType

    nc = bacc.Bacc("TRN2", target_bir_lowering=False)

    qTin = nc.dram_tensor("qT", [D, Q], bf16, kind="ExternalInput")
    refTin = nc.dram_tensor("refT", [D, R], bf16, kind="ExternalInput")
    posTin = nc.dram_tensor("posT", [D, R], bf16, kind="ExternalInput")
    Wq_d = nc.dram_tensor("Wq", [D, D], bf16, kind="ExternalInput")
    Wk_d = nc.dram_tensor("Wk", [D, D], bf16, kind="ExternalInput")
    Wv_d = nc.dram_tensor("Wv", [D, D], bf16, kind="ExternalInput")
    Wr_d = nc.dram_tensor("Wr", [D, D], bf16, kind="ExternalInput")
    Wo_d = nc.dram_tensor("Wo", [D, D], bf16, kind="ExternalInput")
    cb_d = nc.dram_tensor("cb", [128, NI], f32, kind="ExternalInput")
    pb_d = nc.dram_tensor("pb", [128, NI], f32, kind="ExternalInput")
    out_d = nc.dram_tensor("out", [Q, D], f32, kind="ExternalOutput")

    with tile.TileContext(nc) as tc:
        with (
            tc.tile_pool(name="persist", bufs=1) as persist,
            tc.tile_pool(name="mmp", bufs=2, space="PSUM") as mmp,
            tc.tile_pool(name="scp", bufs=3, space="PSUM") as scp,
            tc.tile_pool(name="denp", bufs=3) as denp,
            tc.tile_pool(name="wst", bufs=2) as wst,
        ):
            ident = persist.tile([128, 128], bf16, tag="ident")
            make_identity(nc, ident)
            cb_sb = persist.tile([128, NI], f32, tag="cb")
            pb_sb = persist.tile([128, NI], f32, tag="pb")
            nc.sync.dma_start(out=cb_sb, in_=cb_d[:, :])
            nc.sync.dma_start(out=pb_sb, in_=pb_d[:, :])

            kT = persist.tile([128, NI, R], bf16, tag="kT")
            v_sb = persist.tile([128, NRT, D], bf16, tag="v")
            qcb = persist.tile([128, NI, Q], bf16, tag="qcb")
            qpb = persist.tile([128, NI, Q], bf16, tag="qpb")
            rT = persist.tile([128, NI, R], bf16, tag="rT")
            outT = persist.tile([128, NI, Q], bf16, tag="outT")
            # pst ring: NSLOT slots x 2 heads x PIT (pad cols 1024..1536)
            pst = persist.tile([128, NSLOT, 2, PIT], bf16, tag="pst")
            scratch = persist.tile([128, 512], bf16, tag="scratch")

            # ---- warmup: release the HAM clock gate ----
            nc.vector.memset(scratch, 0.0)
            wps = mmp.tile([128, 512], f32, tag="pw", name="wps")
            for i in range(16):
                nc.tensor.matmul(wps, lhsT=ident, rhs=scratch,
                                 start=(i == 0), stop=(i == 15))
            nc.scalar.copy(scratch, wps)

            # ---- pst init: data zeros + pad PAD_VAL (one-time) ----
            for sl in range(NSLOT):
                for hh in range(2):
                    nc.vector.memset(pst[:, sl, hh, 0:R], 0.0)
                    nc.vector.memset(pst[:, sl, hh, R:PIT], PAD_VAL)

            # ---- helpers ----
            def load_w(w_dram):
                wt = wst.tile([128, NKD, D], bf16, tag="w", name="wt")
                for kd in range(NKD):
                    nc.gpsimd.dma_start(
                        out=wt[:, kd, :],
                        in_=w_dram[kd * 128 : (kd + 1) * 128, :])
                return wt

            def wave(w_sb, i, rhs_of, emit, eng_i):
                ps = mmp.tile([128, 512], f32, tag="pw", name="ps")
                for kd in range(NKD):
                    nc.tensor.matmul(
                        ps,
                        lhsT=w_sb[:, kd, ds(i * 128, 128)],
                        rhs=rhs_of(kd),
                        start=(kd == 0),
                        stop=(kd == NKD - 1))
                emit(ps, eng_i)

            # ---- input loads (gpsimd queue order matters) ----
            with tc.tile_pool(name="inp2", bufs=1) as inp2:
                refT = inp2.tile([128, NKD, R], bf16, tag="refT")

                with tc.tile_pool(name="inp1", bufs=1) as inp1:
                    posT = inp1.tile([128, NKD, R], bf16, tag="posT")
                    qT_sb = inp1.tile([128, NKD, Q], bf16, tag="qTin")
                    for kd in range(NKD):
                        nc.gpsimd.dma_start(
                            out=posT[:, kd, :],
                            in_=posTin[kd * 128 : (kd + 1) * 128, :])
                    for kd in range(NKD):
                        nc.gpsimd.dma_start(
                            out=qT_sb[:, kd, :],
                            in_=qTin[kd * 128 : (kd + 1) * 128, :])

                    Wr_sb = load_w(Wr_d)
                    Wq_sb = load_w(Wq_d)
                    for kd in range(NKD):
                        nc.gpsimd.dma_start(
                            out=refT[:, kd, :],
                            in_=refTin[kd * 128 : (kd + 1) * 128, :])

                    # ---- rT projection ----
                    for nb in range(2):
                        for i in range(NI):
                            def em_rT(ps, e, i=i, nb=nb):
                                if e % 2:
                                    nc.vector.tensor_copy(
                                        rT[:, i, ds(nb * 512, 512)], ps)
                                else:
                                    nc.scalar.copy(
                                        rT[:, i, ds(nb * 512, 512)], ps)
                            wave(Wr_sb, i,
                                 lambda kd, nb=nb: posT[:, kd,
                                                        ds(nb * 512, 512)],
                                 em_rT, nb * NI + i)

                    # ---- q projection (+ biases) ----
                    Wk_sb = load_w(Wk_d)  # prefetch
                    for i in range(NI):
                        def em_q(ps, e, i=i):
                            nc.vector.tensor_scalar_add(
                                qcb[:, i, :], ps, cb_sb[:, i : i + 1])
                            nc.vector.tensor_scalar_add(
                                qpb[:, i, :], ps, pb_sb[:, i : i + 1])
                        wave(Wq_sb, i, lambda kd: qT_sb[:, kd, :], em_q, i)

                # inp1 (posT, qT) closed; C/D pools may open now
                with (
                    tc.tile_pool(name="pshp", bufs=8) as pshp,
                    tc.tile_pool(name="attnp", bufs=4) as attnp,
                    tc.tile_pool(name="attnTp", bufs=3) as attnTp,
                ):
                    # =========== attention pipeline helpers ===========
                    PP = NSLOT * 2 * PIT  # pst per-partition pitch (elems)

                    def emit_C(j, qt):
                        """position logits P for pair j, q-tile qt ->
                        pst ring slot; then skewed SBUF->SBUF reads.
                        NOTE: both copies precede both skews — a pst write
                        emitted after a skew picks up a conservative WAR
                        dep on it (the skew AP spans the whole tensor),
                        which can deadlock against psh-pool recycling."""
                        g = (j * NQT + qt) % NSLOT
                        pps = []
                        for hh in range(2):
                            off = hh * 64
                            pp = scp.tile([128, R], f32, tag="sc", name="pp")
                            for rb in range(2):
                                nc.tensor.matmul(
                                    pp[:, ds(rb * 512, 512)],
                                    lhsT=qpb[off : off + 64, j,
                                             ds(qt * 128, 128)],
                                    rhs=rT[off : off + 64, j,
                                           ds(rb * 512, 512)],
                                    start=True, stop=True,
                                    tile_position=(off, 0))
                            pps.append(pp)
                        for hh in range(2):
                            nc.vector.tensor_copy(pst[:, g, hh, 0:R],
                                                  pps[hh])
                        pshs = []
                        for hh in range(2):
                            # skewed read implementing rel_shift
                            psh = pshp.tile([128, R], bf16, tag="psh",
                                            name="psh")
                            rd = bass_mod.AP(
                                tensor=pst.tensor,
                                offset=pst.offset + (g * 2 + hh) * PIT
                                + 511 - qt * 128,
                                ap=[[PP - 1, 128], [1, R]])
                            nc.gpsimd.dma_start(out=psh, in_=rd)
                            pshs.append(psh)
                        return pshs

                    attn_half = {}  # (j, hh, half) -> tile
                    den_pair = {}  # j -> den tile [128, 8]
                    aT_half = {}  # (j, hh, half) -> attnT tile

                    def emit_D(j, qt, pshs):
                        """scores + exp for (pair j, q-tile qt)."""
                        if qt == 0:
                            den_pair[j] = denp.tile([128, 8], f32, tag="den",
                                                    name="den")
                        half = qt // 2
                        scs = []
                        for hh in range(2):
                            off = hh * 64
                            sc = scp.tile([128, R], f32, tag="sc", name="sc")
                            for rb in range(2):
                                nc.tensor.matmul(
                                    sc[:, ds(rb * 512, 512)],
                                    lhsT=qcb[off : off + 64, j,
                                             ds(qt * 128, 128)],
                                    rhs=kT[off : off + 64, j,
                                           ds(rb * 512, 512)],
                                    start=True, stop=False,
                                    tile_position=(off, 0))
                            scs.append(sc)
                        for hh, sc in enumerate(scs):
                            for rb in range(2):
                                nc.tensor.matmul(
                                    sc[:, ds(rb * 512, 512)],
                                    lhsT=ident,
                                    rhs=pshs[hh][:, ds(rb * 512, 512)],
                                    start=False, stop=True,
                                    skip_group_check=True)
                        for hh, sc in enumerate(scs):
                            if qt % 2 == 0:
                                attn_half[(j, hh, half)] = attnp.tile(
                                    [128, 2, R], bf16, tag="attn",
                                    name="attn")
                            at = attn_half[(j, hh, half)]
                            nc.scalar.activation(
                                at[:, qt % 2, :], sc, AF.Exp, scale=0.125,
                                accum_out=den_pair[j][:, hh * 4 + qt :
                                                      hh * 4 + qt + 1])

                    def emit_norm(j, half):
                        """recip + normalize + transpose for a half."""
                        den = den_pair[j]
                        rec = denp.tile([128, 8], f32, tag="den", name="rec")
                        for hh in range(2):
                            c = hh * 4 + half * 2
                            nc.vector.reciprocal(rec[:, c : c + 2],
                                                 den[:, c : c + 2])
                        for hh in range(2):
                            at = attn_half[(j, hh, half)]
                            for sub in range(2):
                                c = hh * 4 + half * 2 + sub
                                eng = nc.gpsimd if sub == 0 else nc.vector
                                eng.tensor_scalar_mul(
                                    at[:, sub, :], at[:, sub, :],
                                    rec[:, c : c + 1])
                            if half == 0:
                                aT_half[(j, hh)] = attnTp.tile(
                                    [128, NQT * NRT, 128], bf16,
                                    tag="aT", name="aT")
                            aT = aT_half[(j, hh)]
                            nc.sync.dma_start_transpose(
                                aT[:, half * 2 * NRT : (half + 1) * 2 * NRT,
                                   :], at)

                    def emit_PV(j):
                        pv = mmp.tile([128, 512], f32, tag="pw", name="pv")
                        for rt in range(NRT):
                            for hh in range(2):
                                off = hh * 64
                                h = 2 * j + hh
                                aT = aT_half[(j, hh)]
                                a4 = aT.rearrange(
                                    "p (qt rt) q -> p qt rt q", rt=NRT)
                                nc.tensor.matmul(
                                    pv[off : off + 64, :],
                                    lhsT=v_sb[:, rt, ds(h * 64, 64)],
                                    rhs=a4[:, :, rt, :],
                                    start=(rt == 0),
                                    stop=(rt == NRT - 1),
                                    tile_position=(0, off))
                        for hh in range(2):
                            del aT_half[(j, hh)]
                            del attn_half[(j, hh, 0)]
                            del attn_half[(j, hh, 1)]
                        nc.vector.tensor_copy(outT[:, j, :], pv)

                    # =========== fused schedule ===========
                    # C(0) standalone (needs only rT/qpb slice 0)
                    psh_q = {}
                    for qt in range(NQT):
                        psh_q[(0, qt)] = emit_C(0, qt)

                    # kT projection i-outer (pair j needs only slice i=j),
                    # C(1) and D(0) interleaved so psh/scp rings recycle
                    def em_kT(ps, e, i, nb):
                        if e % 2:
                            nc.vector.tensor_copy(
                                kT[:, i, ds(nb * 512, 512)], ps)
                        else:
                            nc.scalar.copy(kT[:, i, ds(nb * 512, 512)], ps)

                    for i in range(NI):
                        for nb in range(2):
                            wave(Wk_sb, i,
                                 lambda kd, nb=nb: refT[:, kd,
                                                        ds(nb * 512, 512)],
                                 lambda ps, e, i=i, nb=nb: em_kT(ps, e, i,
                                                                 nb),
                                 2 * i + nb)
                        if i % 2 == 0:
                            psh_q[(1, i // 2)] = emit_C(1, i // 2)
                        else:
                            qt = (i - 1) // 2
                            emit_D(0, qt, psh_q.pop((0, qt)))
                            if qt % 2 == 1:
                                emit_norm(0, qt // 2)

                    Wv_sb = load_w(Wv_d)
                    Wo_sb = load_w(Wo_d)

                    # pair loop; v projection fully inside j=1 (PV(0) at
                    # the end of j=1 needs all of v)
                    vw = 0  # v waves emitted (16 total: nb*8 + rt)

                    def v_wave():
                        nonlocal vw
                        if vw >= 16:
                            return
                        nb, rt = vw // NRT, vw % NRT
                        ps = mmp.tile([128, 512], f32, tag="pw", name="ps")
                        for kd in range(NKD):
                            nc.tensor.matmul(
                                ps,
                                lhsT=refT[:, kd, ds(rt * 128, 128)],
                                rhs=Wv_sb[:, kd, ds(nb * 512, 512)],
                                start=(kd == 0),
                                stop=(kd == NKD - 1))
                        if vw % 2:
                            nc.vector.tensor_copy(
                                v_sb[:, rt, ds(nb * 512, 512)], ps)
                        else:
                            nc.scalar.copy(
                                v_sb[:, rt, ds(nb * 512, 512)], ps)
                        vw += 1

                    for j in range(1, NP):
                        for qt in range(NQT):
                            if j + 1 < NP:
                                psh_q[(j + 1, qt)] = emit_C(j + 1, qt)
                            emit_D(j, qt, psh_q.pop((j, qt)))
                            if qt % 2 == 1:
                                emit_norm(j, qt // 2)
                            if j == 1:
                                for _ in range(4):
                                    v_wave()
                        emit_PV(j - 1)
                        del den_pair[j - 1]
                    emit_PV(NP - 1)
                    del den_pair[NP - 1]

                # ---- output projection ----
                with tc.tile_pool(name="ost", bufs=3) as ostp:
                    for qt in range(NQT):
                        for db in range(2):
                            op = mmp.tile([128, 512], f32, tag="pw",
                                          name="op")
                            for i in range(NI):
                                nc.tensor.matmul(
                                    op,
                                    lhsT=outT[:, i, ds(qt * 128, 128)],
                                    rhs=Wo_sb[:, i, ds(db * 512, 512)],
                                    start=(i == 0),
                                    stop=(i == NI - 1))
                            ot = ostp.tile([128, 512], f32, tag="ot",
                                           name="ot")
                            if (qt + db) % 2:
                                nc.vector.tensor_copy(ot, op)
                            else:
                                nc.scalar.copy(ot, op)
                            nc.sync.dma_start(
                                out=out_d[qt * 128 : (qt + 1) * 128,
                                          db * 512 : (db + 1) * 512],
                                in_=ot)

    return nc


def _get_nc():
    if "nc" not in _CACHE:
        nc = _build_nc()
        if not nc.is_finalized():
            nc.finalize()
        _CACHE["nc"] = nc
    return _CACHE["nc"]


def _prep_in_maps(inputs):
    import ml_dtypes

    bf = ml_dtypes.bfloat16
    q = np.asarray(inputs["query_seqs"], dtype=np.float32)
    mem = np.asarray(inputs["memory_seqs"], dtype=np.float32)
    pos = np.asarray(inputs["positional_encoding"], dtype=np.float32)
    Wq = np.asarray(inputs["Wq"], dtype=np.float32).reshape(D, D).astype(bf)
    Wk = np.asarray(inputs["Wk"], dtype=np.float32).reshape(D, D).astype(bf)
    Wv = np.asarray(inputs["Wv"], dtype=np.float32).reshape(D, D).astype(bf)
    Wr = np.asarray(inputs["Wr"], dtype=np.float32).reshape(D, D).astype(bf)
    Wo = np.asarray(inputs["Wo"], dtype=np.float32).reshape(D, D).astype(bf)
    cb = np.ascontiguousarray(
        np.asarray(inputs["content_bias"], dtype=np.float32)
        .reshape(D).reshape(NI, 128).T)
    pb = np.ascontiguousarray(
        np.asarray(inputs["position_bias"], dtype=np.float32)
        .reshape(D).reshape(NI, 128).T)
    posT = np.ascontiguousarray(pos.T).astype(bf)

    in_maps = []
    for b in range(B):
        refT = np.ascontiguousarray(
            np.concatenate([mem[b], q[b]], axis=0).T).astype(bf)
        qT = np.ascontiguousarray(q[b].T).astype(bf)
        in_maps.append(
            dict(qT=qT, refT=refT, posT=posT,
                 Wq=Wq, Wk=Wk, Wv=Wv, Wr=Wr, Wo=Wo, cb=cb, pb=pb))
    return in_maps


def run_spmd(inputs, **kwargs):
    """Run on 8 cores; returns (output [B,Q,D], BassKernelResults)."""
    from concourse.bass_utils import run_bass_kernel_spmd

    nc = _get_nc()
    in_maps = _prep_in_maps(inputs)
    res = run_bass_kernel_spmd(nc, in_maps, core_ids=list(range(B)), **kwargs)
    out = np.stack([r["out"] for r in res.results], axis=0).astype(np.float32)
    return out, res


def kernel(**inputs) -> np.ndarray:
    out, _ = run_spmd(inputs)
    return out


# revision 13
# speedup vs baseline: 1.9677x; 1.9677x over previous
"""TransformerXL relative attention on 8 TRN2 NeuronCores (batch-parallel).

v3: fully fused single-pass pipeline, rel_shift via skewed SBUF->SBUF DMA
(no DRAM round trip), PE kept HAM-warm end to end.

Per-core (one batch element):
  - warmup matmul burst releases the HAM clock gate before real work
  - projections (rT, qcb/qpb, kT, v) as 8-matmul PSUM waves; kT and v are
    interleaved with the attention pair pipeline
  - position logits P per (pair, head, q-tile) -> PSUM -> bf16 pst ring
    (rows padded to 1536 with -30000); rel_shift read back with a skewed
    flat AP (partition stride = row_pitch - 1) SBUF->SBUF
  - scores = content matmul + identity-matmul(P_shifted); Exp on ACT with
    accum_out denominators; batched reciprocals; normalize on DVE;
    attn transposed via DMA-xbar (sync queue) to attnT [r, q]
  - PV matmuls (col-packed head pairs), output projection tail
"""

import sys

if "/opt/trn_rl_repo" not in sys.path:
    sys.path.insert(0, "/opt/trn_rl_repo")

import numpy as np

B, Q, MEM, D, H, S = 8, 512, 512, 1024, 16, 64
R = Q + MEM  # 1024
PIT = 1536  # padded row pitch of a pst slot (1024 data + 512 pad)
PAD_VAL = -30000.0
NKD = D // 128  # 8 contraction tiles
NI = D // 128  # 8 hs-tiles
NQT = Q // 128  # 4 q-tiles
NRT = R // 128  # 8 r-tiles
NP = H // 2  # 8 head pairs
NSLOT = 4  # pst ring slots

_CACHE = {}


def _build_nc():
    import concourse.bass as bass_mod
    import concourse.mybir as mybir
    import concourse.tile as tile
    from concourse import bacc
    from concourse.bass import ds
    from concourse.masks import make_identity

    f32 = mybir.dt.float32
    bf16 = mybir.dt.bfloat16
    AF = mybir.ActivationFunctionType

    nc = bacc.Bacc("TRN2", target_bir_lowering=False)

    qTin = nc.dram_tensor("qT", [D, Q], bf16, kind="ExternalInput")
    refTin = nc.dram_tensor("refT", [D, R], bf16, kind="ExternalInput")
    posTin = nc.dram_tensor("posT", [D, R], bf16, kind="ExternalInput")
    Wq_d = nc.dram_tensor("Wq", [D, D], bf16, kind="ExternalInput")
    Wk_d = nc.dram_tensor("Wk", [D, D], bf16, kind="ExternalInput")
    Wv_d = nc.dram_tensor("Wv", [D, D], bf16, kind="ExternalInput")
    Wr_d = nc.dram_tensor("Wr", [D, D], bf16, kind="ExternalInput")
    Wo_d = nc.dram_tensor("Wo", [D, D], bf16, kind="ExternalInput")
    cb_d = nc.dram_tensor("cb", [128, NI], f32, kind="ExternalInput")
    pb_d = nc.dram_tensor("pb", [128, NI], f32, kind="ExternalInput")
    out_d = nc.dram_tensor("out", [Q, D], f32, kind="ExternalOutput")

    with tile.TileContext(nc) as tc:
        with (
            tc.tile_pool(name="persist", bufs=1) as persist,
            tc.tile_pool(name="mmp", bufs=2, space="PSUM") as mmp,
            tc.tile_pool(name="scp", bufs=3, space="PSUM") as scp,
            tc.tile_pool(name="denp", bufs=3) as denp,
            tc.tile_pool(name="wst", bufs=2) as wst,
        ):
            ident = persist.tile([128, 128], bf16, tag="ident")
            make_identity(nc, ident)
            cb_sb = persist.tile([128, NI], f32, tag="cb")
            pb_sb = persist.tile([128, NI], f32, tag="pb")
            nc.sync.dma_start(out=cb_sb, in_=cb_d[:, :])
            nc.sync.dma_start(out=pb_sb, in_=pb_d[:, :])

            kT = persist.tile([128, NI, R], bf16, tag="kT")
            v_sb = persist.tile([128, NRT, D], bf16, tag="v")
            qcb = persist.tile([128, NI, Q], bf16, tag="qcb")
            qpb = persist.tile([128, NI, Q], bf16, tag="qpb")
            rT = persist.tile([128, NI, R], bf16, tag="rT")
            outT = persist.tile([128, NI, Q], bf16, tag="outT")
            # pst ring: NSLOT slots x 2 heads x PIT (pad cols 1024..1536)
            pst = persist.tile([128, NSLOT, 2, PIT], bf16, tag="pst")
            scratch = persist.tile([128, 512], bf16, tag="scratch")

            # ---- warmup: release the HAM clock gate ----
            nc.vector.memset(scratch, 0.0)
            wps = mmp.tile([128, 512], f32, tag="pw", name="wps")
            for i in range(16):
                nc.tensor.matmul(wps, lhsT=ident, rhs=scratch,
                                 start=(i == 0), stop=(i == 15))
            nc.scalar.copy(scratch, wps)

            # ---- pst init: data zeros + pad PAD_VAL (one-time) ----
            for sl in range(NSLOT):
                for hh in range(2):
                    nc.vector.memset(pst[:, sl, hh, 0:R], 0.0)
                    nc.vector.memset(pst[:, sl, hh, R:PIT], PAD_VAL)

            # ---- helpers ----
            def load_w(w_dram):
                wt = wst.tile([128, NKD, D], bf16, tag="w", name="wt")
                for kd in range(NKD):
                    nc.gpsimd.dma_start(
                        out=wt[:, kd, :],
                        in_=w_dram[kd * 128 : (kd + 1) * 128, :])
                return wt

            def wave(w_sb, i, rhs_of, emit, eng_i):
                ps = mmp.tile([128, 512], f32, tag="pw", name="ps")
                for kd in range(NKD):
                    nc.tensor.matmul(
                        ps,
                        lhsT=w_sb[:, kd, ds(i * 128, 128)],
                        rhs=rhs_of(kd),
                        start=(kd == 0),
                        stop=(kd == NKD - 1))
                emit(ps, eng_i)

            # ---- input loads (gpsimd queue order matters) ----
            with tc.tile_pool(name="inp2", bufs=1) as inp2:
                refT = inp2.tile([128, NKD, R], bf16, tag="refT")

                with tc.tile_pool(name="inp1", bufs=1) as inp1:
                    posT = inp1.tile([128, NKD, R], bf16, tag="posT")
                    qT_sb = inp1.tile([128, NKD, Q], bf16, tag="qTin")
                    for kd in range(NKD):
                        nc.gpsimd.dma_start(
                            out=posT[:, kd, :],
                            in_=posTin[kd * 128 : (kd + 1) * 128, :])
                    for kd in range(NKD):
                        nc.gpsimd.dma_start(
                            out=qT_sb[:, kd, :],
                            in_=qTin[kd * 128 : (kd + 1) * 128, :])

                    Wr_sb = load_w(Wr_d)
                    Wq_sb = load_w(Wq_d)
                    for kd in range(NKD):
                        nc.gpsimd.dma_start(
                            out=refT[:, kd, :],
                            in_=refTin[kd * 128 : (kd + 1) * 128, :])

                    # ---- rT projection ----
                    for nb in range(2):
                        for i in range(NI):
                            def em_rT(ps, e, i=i, nb=nb):
                                if e % 2:
                                    nc.vector.tensor_copy(
                                        rT[:, i, ds(nb * 512, 512)], ps)
                                else:
                                    nc.scalar.copy(
                                        rT[:, i, ds(nb * 512, 512)], ps)
                            wave(Wr_sb, i,
                                 lambda kd, nb=nb: posT[:, kd,
                                                        ds(nb * 512, 512)],
                                 em_rT, nb * NI + i)

                    # ---- q projection (+ biases) ----
                    Wk_sb = load_w(Wk_d)  # prefetch
                    for i in range(NI):
                        def em_q(ps, e, i=i):
                            nc.vector.tensor_scalar_add(
                                qcb[:, i, :], ps, cb_sb[:, i : i + 1])
                            nc.vector.tensor_scalar_add(
                                qpb[:, i, :], ps, pb_sb[:, i : i + 1])
                        wave(Wq_sb, i, lambda kd: qT_sb[:, kd, :], em_q, i)

                # inp1 (posT, qT) closed; C/D pools may open now
                with (
                    tc.tile_pool(name="pshp", bufs=8) as pshp,
                    tc.tile_pool(name="attnp", bufs=4) as attnp,
                    tc.tile_pool(name="attnTp", bufs=3) as attnTp,
                ):
                    # =========== attention pipeline helpers ===========
                    PP = NSLOT * 2 * PIT  # pst per-partition pitch (elems)

                    def emit_C(j, qt):
                        """position logits P for pair j, q-tile qt ->
                        pst ring slot; then skewed SBUF->SBUF reads.
                        NOTE: both copies precede both skews — a pst write
                        emitted after a skew picks up a conservative WAR
                        dep on it (the skew AP spans the whole tensor),
                        which can deadlock against psh-pool recycling."""
                        g = (j * NQT + qt) % NSLOT
                        pps = []
                        for hh in range(2):
                            off = hh * 64
                            pp = scp.tile([128, R], f32, tag="sc", name="pp")
                            for rb in range(2):
                                nc.tensor.matmul(
                                    pp[:, ds(rb * 512, 512)],
                                    lhsT=qpb[off : off + 64, j,
                                             ds(qt * 128, 128)],
                                    rhs=rT[off : off + 64, j,
                                           ds(rb * 512, 512)],
                                    start=True, stop=True,
                                    tile_position=(off, 0))
                            pps.append(pp)
                        for hh in range(2):
                            if (j * NQT + qt + hh) % 4 == 3:
                                nc.scalar.copy(pst[:, g, hh, 0:R], pps[hh])
                            else:
                                nc.vector.tensor_copy(pst[:, g, hh, 0:R],
                                                      pps[hh])
                        pshs = []
                        for hh in range(2):
                            # skewed read implementing rel_shift
                            psh = pshp.tile([128, R], bf16, tag="psh",
                                            name="psh")
                            rd = bass_mod.AP(
                                tensor=pst.tensor,
                                offset=pst.offset + (g * 2 + hh) * PIT
                                + 511 - qt * 128,
                                ap=[[PP - 1, 128], [1, R]])
                            nc.gpsimd.dma_start(out=psh, in_=rd)
                            pshs.append(psh)
                        return pshs

                    attn_half = {}  # (j, hh, half) -> tile
                    den_pair = {}  # j -> den tile [128, 8]
                    aT_half = {}  # (j, hh, half) -> attnT tile

                    def emit_D(j, qt, pshs):
                        """scores + exp for (pair j, q-tile qt)."""
                        if qt == 0:
                            den_pair[j] = denp.tile([128, 8], f32, tag="den",
                                                    name="den")
                        half = qt // 2
                        scs = []
                        for hh in range(2):
                            off = hh * 64
                            sc = scp.tile([128, R], f32, tag="sc", name="sc")
                            for rb in range(2):
                                nc.tensor.matmul(
                                    sc[:, ds(rb * 512, 512)],
                                    lhsT=qcb[off : off + 64, j,
                                             ds(qt * 128, 128)],
                                    rhs=kT[off : off + 64, j,
                                           ds(rb * 512, 512)],
                                    start=True, stop=False,
                                    tile_position=(off, 0))
                            scs.append(sc)
                        for hh, sc in enumerate(scs):
                            for rb in range(2):
                                nc.tensor.matmul(
                                    sc[:, ds(rb * 512, 512)],
                                    lhsT=ident,
                                    rhs=pshs[hh][:, ds(rb * 512, 512)],
                                    start=False, stop=True,
                                    skip_group_check=True)
                        for hh, sc in enumerate(scs):
                            if qt % 2 == 0:
                                attn_half[(j, hh, half)] = attnp.tile(
                                    [128, 2, R], bf16, tag="attn",
                                    name="attn")
                            at = attn_half[(j, hh, half)]
                            nc.scalar.activation(
                                at[:, qt % 2, :], sc, AF.Exp, scale=0.125,
                                accum_out=den_pair[j][:, hh * 4 + qt :
                                                      hh * 4 + qt + 1])

                    def emit_norm(j, half):
                        """recip + normalize + transpose for a half."""
                        den = den_pair[j]
                        rec = denp.tile([128, 8], f32, tag="den", name="rec")
                        for hh in range(2):
                            c = hh * 4 + half * 2
                            nc.vector.reciprocal(rec[:, c : c + 2],
                                                 den[:, c : c + 2])
                        for hh in range(2):
                            at = attn_half[(j, hh, half)]
                            for sub in range(2):
                                c = hh * 4 + half * 2 + sub
                                nc.vector.tensor_scalar_mul(
                                    at[:, sub, :], at[:, sub, :],
                                    rec[:, c : c + 1])
                            if half == 0:
                                aT_half[(j, hh)] = attnTp.tile(
                                    [128, NQT * NRT, 128], bf16,
                                    tag="aT", name="aT")
                            aT = aT_half[(j, hh)]
                            nc.sync.dma_start_transpose(
                                aT[:, half * 2 * NRT : (half + 1) * 2 * NRT,
                                   :], at)

                    def emit_PV(j):
                        pv = mmp.tile([128, 512], f32, tag="pw", name="pv")
                        for rt in range(NRT):
                            for hh in range(2):
                                off = hh * 64
                                h = 2 * j + hh
                                aT = aT_half[(j, hh)]
                                a4 = aT.rearrange(
                                    "p (qt rt) q -> p qt rt q", rt=NRT)
                                nc.tensor.matmul(
                                    pv[off : off + 64, :],
                                    lhsT=v_sb[:, rt, ds(h * 64, 64)],
                                    rhs=a4[:, :, rt, :],
                                    start=(rt == 0),
                                    stop=(rt == NRT - 1),
                                    tile_position=(0, off))
                        for hh in range(2):
                            del aT_half[(j, hh)]
                            del attn_half[(j, hh, 0)]
                            del attn_half[(j, hh, 1)]
                        nc.vector.tensor_copy(outT[:, j, :], pv)

                    # =========== fused schedule ===========
                    # C(0) standalone (needs only rT/qpb slice 0)
                    psh_q = {}
                    for qt in range(NQT):
                        psh_q[(0, qt)] = emit_C(0, qt)

                    # kT projection i-outer (pair j needs only slice i=j),
                    # C(1) and D(0) interleaved so psh/scp rings recycle
                    def em_kT(ps, e, i, nb):
                        if e % 2:
                            nc.vector.tensor_copy(
                                kT[:, i, ds(nb * 512, 512)], ps)
                        else:
                            nc.scalar.copy(kT[:, i, ds(nb * 512, 512)], ps)

                    for i in range(NI):
                        for nb in range(2):
                            wave(Wk_sb, i,
                                 lambda kd, nb=nb: refT[:, kd,
                                                        ds(nb * 512, 512)],
                                 lambda ps, e, i=i, nb=nb: em_kT(ps, e, i,
                                                                 nb),
                                 2 * i + nb)
                        if i % 2 == 0:
                            psh_q[(1, i // 2)] = emit_C(1, i // 2)
                        else:
                            qt = (i - 1) // 2
                            emit_D(0, qt, psh_q.pop((0, qt)))
                            if qt % 2 == 1:
                                emit_norm(0, qt // 2)

                    Wv_sb = load_w(Wv_d)
                    Wo_sb = load_w(Wo_d)

                    # pair loop; v projection fully inside j=1 (PV(0) at
                    # the end of j=1 needs all of v)
                    vw = 0  # v waves emitted (16 total: nb*8 + rt)

                    def v_wave():
                        nonlocal vw
                        if vw >= 16:
                            return
                        nb, rt = vw // NRT, vw % NRT
                        ps = mmp.tile([128, 512], f32, tag="pw", name="ps")
                        for kd in range(NKD):
                            nc.tensor.matmul(
                                ps,
                                lhsT=refT[:, kd, ds(rt * 128, 128)],
                                rhs=Wv_sb[:, kd, ds(nb * 512, 512)],
                                start=(kd == 0),
                                stop=(kd == NKD - 1))
                        if vw % 2:
                            nc.vector.tensor_copy(
                                v_sb[:, rt, ds(nb * 512, 512)], ps)
                        else:
                            nc.scalar.copy(
                                v_sb[:, rt, ds(nb * 512, 512)], ps)
                        vw += 1

                    for j in range(1, NP):
                        for qt in range(NQT):
                            if j + 1 < NP:
                                psh_q[(j + 1, qt)] = emit_C(j + 1, qt)
                            emit_D(j, qt, psh_q.pop((j, qt)))
                            if qt % 2 == 1:
                                emit_norm(j, qt // 2)
                            if j == 1:
                                for _ in range(4):
                                    v_wave()
                        emit_PV(j - 1)
                        del den_pair[j - 1]
                    emit_PV(NP - 1)
                    del den_pair[NP - 1]

                # ---- output projection ----
                with tc.tile_pool(name="ost", bufs=3) as ostp:
                    for qt in range(NQT):
                        for db in range(2):
                            op = mmp.tile([128, 512], f32, tag="pw",
                                          name="op")
                            for i in range(NI):
                                nc.tensor.matmul(
                                    op,
                                    lhsT=outT[:, i, ds(qt * 128, 128)],
                                    rhs=Wo_sb[:, i, ds(db * 512, 512)],
                                    start=(i == 0),
                                    stop=(i == NI - 1))
                            ot = ostp.tile([128, 512], f32, tag="ot",
                                           name="ot")
                            if (qt + db) % 2:
                                nc.vector.tensor_copy(ot, op)
                            else:
                                nc.scalar.copy(ot, op)
                            nc.sync.dma_start(
                                out=out_d[qt * 128 : (qt + 1) * 128,
                                          db * 512 : (db + 1) * 512],
                                in_=ot)

    return nc


def _get_nc():
    if "nc" not in _CACHE:
        nc = _build_nc()
        if not nc.is_finalized():
            nc.finalize()
        _CACHE["nc"] = nc
    return _CACHE["nc"]


def _prep_in_maps(inputs):
    import ml_dtypes

    bf = ml_dtypes.bfloat16
    q = np.asarray(inputs["query_seqs"], dtype=np.float32)
    mem = np.asarray(inputs["memory_seqs"], dtype=np.float32)
    pos = np.asarray(inputs["positional_encoding"], dtype=np.float32)
    Wq = np.asarray(inputs["Wq"], dtype=np.float32).reshape(D, D).astype(bf)
    Wk = np.asarray(inputs["Wk"], dtype=np.float32).reshape(D, D).astype(bf)
    Wv = np.asarray(inputs["Wv"], dtype=np.float32).reshape(D, D).astype(bf)
    Wr = np.asarray(inputs["Wr"], dtype=np.float32).reshape(D, D).astype(bf)
    Wo = np.asarray(inputs["Wo"], dtype=np.float32).reshape(D, D).astype(bf)
    cb = np.ascontiguousarray(
        np.asarray(inputs["content_bias"], dtype=np.float32)
        .reshape(D).reshape(NI, 128).T)
    pb = np.ascontiguousarray(
        np.asarray(inputs["position_bias"], dtype=np.float32)
        .reshape(D).reshape(NI, 128).T)
    posT = np.ascontiguousarray(pos.T).astype(bf)

    in_maps = []
    for b in range(B):
        refT = np.ascontiguousarray(
            np.concatenate([mem[b], q[b]], axis=0).T).astype(bf)
        qT = np.ascontiguousarray(q[b].T).astype(bf)
        in_maps.append(
            dict(qT=qT, refT=refT, posT=posT,
                 Wq=Wq, Wk=Wk, Wv=Wv, Wr=Wr, Wo=Wo, cb=cb, pb=pb))
    return in_maps


def run_spmd(inputs, **kwargs):
    """Run on 8 cores; returns (output [B,Q,D], BassKernelResults)."""
    from concourse.bass_utils import run_bass_kernel_spmd

    nc = _get_nc()
    in_maps = _prep_in_maps(inputs)
    res = run_bass_kernel_spmd(nc, in_maps, core_ids=list(range(B)), **kwargs)
    out = np.stack([r["out"] for r in res.results], axis=0).astype(np.float32)
    return out, res


def kernel(**inputs) -> np.ndarray:
    out, _ = run_spmd(inputs)
    return out
